# revision 1
# baseline (speedup 1.0000x reference)
import os
import sys

sys.path.insert(0, "/opt/trn_rl_repo")

import numpy as np
from concourse import bass, mybir
from concourse.bass_utils import run_bass_kernel_spmd

# nn_PixelConv: feature (8,64,128,128) f32, kernel (8,36,128,128) f32
# -> out (8,64,256,256) f32.  out[n,c,2h+r,2w+q] =
#   sum_{dx,dy in 0..2} F[n,c,h+dy-1,w+dx-1] * K[n,(dx*3+dy)*4+r*2+q,h,w]
# Sharding: pure data-parallel, batch n -> core n.
#
# Device layout (per core): partition dim = w (128).
#   F3  [w, sh=3, ch=32, hc=260]  fp16; F3[w,sh,ch,2hi+cl] = Fpad[2ch+cl, hi, w+sh]
#       (hi in 0..129 covers h=-1..128; the 3 w-shifts are host-precomputed
#        because DVE lanes cannot read across partitions)
#   K2  [w, kk=9, s=4, 256] fp16; K value duplicated over fp16 pairs so the
#       innermost AP step stays +1 (keeps the 2x packed DVE mode)
#   acc [w, s=4, ch=32, 256] fp16 accumulators, DMA'd out raw; host does the
#       pixel shuffle + fp32 cast.

N, C, H, W = 8, 64, 128, 128
LAST_EXEC_TIME_NS = None


def _build_program():
    nc = bass.Bass()
    f16 = mybir.dt.float16
    f3_ext = nc.dram_tensor("f3", [128, 3, 32, 260], f16, kind="ExternalInput")
    k2_ext = nc.dram_tensor("k2", [128, 9, 4, 256], f16, kind="ExternalInput")
    out_ext = nc.dram_tensor("out", [128, 4, 32, 256], f16, kind="ExternalOutput")

    with (
        nc.Block() as block,
        nc.semaphore("dsem") as dsem,
        nc.semaphore("vsem") as vsem,
        nc.sbuf_tensor([128, 3, 32, 260], f16) as f3_sb,
        nc.sbuf_tensor([128, 9, 4, 256], f16) as k2_sb,
        nc.sbuf_tensor([128, 4, 32, 256], f16) as acc_sb,
        nc.sbuf_tensor([128, 32, 256], f16) as prod_sb,
    ):

        @block.sync
        def _(sync):
            sync.dma_start(out=f3_sb[:], in_=f3_ext[:]).then_inc(dsem, 16)
            sync.dma_start(out=k2_sb[:], in_=k2_ext[:]).then_inc(dsem, 16)
            sync.wait_ge(vsem, 1)
            sync.dma_start(out=out_ext[:], in_=acc_sb[:]).then_inc(dsem, 16)

        @block.vector
        def _(v):
            v.wait_ge(dsem, 32)
            last = None
            for s in range(4):
                for kk in range(9):
                    dx, dy = kk // 3, kk % 3
                    in0 = f3_sb[:, dx, :, 2 * dy : 2 * dy + 256]
                    in1 = (
                        k2_sb[:, kk, s, :]
                        .unsqueeze(1)
                        .broadcast_to([128, 32, 256])
                    )
                    if kk == 0:
                        last = v.tensor_tensor(
                            out=acc_sb[:, s], in0=in0, in1=in1,
                            op=mybir.AluOpType.mult,
                        )
                    else:
                        v.tensor_tensor(
                            out=prod_sb[:], in0=in0, in1=in1,
                            op=mybir.AluOpType.mult,
                        )
                        last = v.tensor_tensor(
                            out=acc_sb[:, s], in0=acc_sb[:, s], in1=prod_sb[:],
                            op=mybir.AluOpType.add,
                        )
            last.then_inc(vsem, 1)

    return nc


_NC = None
_HOOK_DONE = False


def _install_ntff_hook():
    # bass_utils' trace path fetches the NTFF profile hook via
    # antenv.axon_hooks, which this image lacks. Install a shim and
    # register the ctypes-based hook (mirrors trn_boot.boot()).
    global _HOOK_DONE
    if _HOOK_DONE:
        return
    _HOOK_DONE = True
    try:
        import antenv.axon_hooks  # noqa: F401

        return
    except ImportError:
        pass
    try:
        import contextlib
        import ctypes
        import types

        import antenv

        mod = types.ModuleType("antenv.axon_hooks")
        holder = {"hook": None}
        mod.set_axon_ntff_profile_hook = lambda h: holder.__setitem__("hook", h)
        mod.get_axon_ntff_profile_hook = lambda: holder["hook"]
        sys.modules["antenv.axon_hooks"] = mod
        antenv.axon_hooks = mod

        lib = ctypes.CDLL("/opt/axon/libaxon_pjrt.so")
        if not hasattr(lib, "axon_start_nrt_profile"):
            return
        lib.axon_start_nrt_profile.argtypes = [
            ctypes.POINTER(ctypes.c_int64),
            ctypes.c_size_t,
        ]
        lib.axon_start_nrt_profile.restype = ctypes.c_int64
        lib.axon_stop_nrt_profile.argtypes = [ctypes.c_char_p]
        lib.axon_stop_nrt_profile.restype = ctypes.c_int64

        @contextlib.contextmanager
        def _hook(output_dir, device_ids):
            import jax

            jax.devices()
            if device_ids:
                ids = (ctypes.c_int64 * len(device_ids))(*device_ids)
                rc = lib.axon_start_nrt_profile(ids, len(device_ids))
            else:
                rc = lib.axon_start_nrt_profile(None, 0)
            if rc != 0:
                raise RuntimeError(f"axon_start_nrt_profile rc={rc}")
            try:
                yield
            finally:
                n = lib.axon_stop_nrt_profile(str(output_dir).encode())
                if n < 0:
                    raise RuntimeError(f"axon_stop_nrt_profile rc={n}")

        mod.set_axon_ntff_profile_hook(_hook)

        from concourse import bass_utils as _bu

        _bu.upload_artifacts = lambda tmpdir: "local://" + str(tmpdir)
    except Exception:
        pass


def _prep_core_inputs(feat_n, kern_n):
    Fp = np.pad(feat_n, ((0, 0), (1, 1), (1, 1)))  # (64,130,130)
    sw = np.lib.stride_tricks.sliding_window_view(Fp, 3, axis=2)  # (c,hi,w,sh)
    f3 = sw.transpose(2, 3, 0, 1)  # (w,sh,c,hi)
    f3 = f3.reshape(128, 3, 32, 2, 130).transpose(0, 1, 2, 4, 3)  # (w,sh,ch,hi,cl)
    f3 = np.ascontiguousarray(f3).astype(np.float16).reshape(128, 3, 32, 260)
    kr = kern_n.reshape(9, 4, 128, 128).transpose(3, 0, 1, 2)  # (w,kk,s,h)
    k2 = np.stack([kr, kr], axis=-1).astype(np.float16).reshape(128, 9, 4, 256)
    return {"f3": f3, "k2": k2}


def _assemble_output(raw_out):
    # raw_out: (128w, 4s, 32ch, 256=2h+cl) fp16
    o = raw_out.astype(np.float32).reshape(128, 4, 32, 128, 2)  # (w,s,ch,h,cl)
    o = o.transpose(2, 4, 3, 0, 1).reshape(64, 128, 128, 4)  # (c,h,w,s)
    o = o.reshape(64, 128, 128, 2, 2).transpose(0, 1, 3, 2, 4)  # (c,h,r,w,q)
    return o.reshape(64, 256, 256)


def kernel(feature: np.ndarray, kernel: np.ndarray) -> np.ndarray:
    global _NC, LAST_EXEC_TIME_NS
    if _NC is None:
        _NC = _build_program()
    feature = np.asarray(feature, dtype=np.float32)
    kernel = np.asarray(kernel, dtype=np.float32)
    in_maps = [_prep_core_inputs(feature[n], kernel[n]) for n in range(N)]
    trace = os.environ.get("PIXELCONV_TRACE", "") not in ("", "0")
    if trace:
        _install_ntff_hook()
    res = run_bass_kernel_spmd(
        _NC, in_maps, core_ids=list(range(N)), trace=trace
    )
    LAST_EXEC_TIME_NS = getattr(res, "exec_time_ns", None)
    out = np.stack([_assemble_output(res.results[n]["out"]) for n in range(N)])
    return out.astype(np.float32)



# revision 6
# speedup vs baseline: 1.9003x; 1.9003x over previous
import os
import sys

sys.path.insert(0, "/opt/trn_rl_repo")

import numpy as np
from concourse import bass, mybir
from concourse.bass_utils import run_bass_kernel_spmd

# nn_PixelConv: feature (8,64,128,128) f32, kernel (8,36,128,128) f32
# -> out (8,64,256,256) f32.  out[n,c,2h+r,2w+q] =
#   sum_{dx,dy in 0..2} F[n,c,h+dy-1,w+dx-1] * K[n,(dx*3+dy)*4+r*2+q,h,w]
# Sharding: pure data-parallel, batch n -> core n.
#
# PE formulation: per pixel px, out[c, s] = sum_t patch[px, t, c] * K[px, t, s]
# (t = dx*3+dy, 9 taps; s = r*2+q, 4 subpixels).  Batch 14 pixels per matmul
# as a block-diagonal stationary:
#   lhsT[K=126=(i,t), M=56=(i,s)] = K-vals on the 14 diagonal 9x4 blocks
#   rhs [K=126,        N=64=c   ] = im2col patches (host-gathered)
#   out [M=56,         N=64    ]  = PSUM fp32, drained to f16 by DVE+Act.
# 16384 px -> G=1184 groups (192 zero-padded px), chunks of 32 groups,
# double-buffered PSUM, deep-buffered input DMA, per-chunk output DMA.
#
# DMA completion sems are per buffer slot: a single counting sem is
# ambiguous when several transfers are in flight (each incs 16 via
# independent per-engine sub-increments, so a threshold can be reached
# with an older transfer still incomplete).  Slot sems only ever carry
# increments from rounds <= the awaited one (issue order is gated on
# consumer progress), so their thresholds are exact.

N = 8
GPC = 14          # pixels per matmul group
TAPS = 9
KDIM = GPC * TAPS  # 126
MDIM = GPC * 4     # 56
GROUPS = 1184      # 37 * 32; 1184*14 = 16576 >= 16384
CHUNK = 32
NCHUNK = GROUPS // CHUNK  # 37
NBUF_IN = 4
NBUF_OB = 4

LAST_EXEC_TIME_NS = None

f16 = mybir.dt.float16
f32 = mybir.dt.float32


def _build_program():
    nc = bass.Bass()
    mv_ext = nc.dram_tensor("mv", [KDIM, NCHUNK, CHUNK, 64], f16, kind="ExternalInput")
    kb_ext = nc.dram_tensor("kb", [KDIM, NCHUNK, CHUNK, MDIM], f16, kind="ExternalInput")
    o_ext = nc.dram_tensor("o", [MDIM, NCHUNK, CHUNK, 64], f16, kind="ExternalOutput")

    import contextlib

    with contextlib.ExitStack() as stack:
        block = stack.enter_context(nc.Block())
        tsem = stack.enter_context(nc.semaphore("tsem"))
        vsem = stack.enter_context(nc.semaphore("vsem"))
        ssem = stack.enter_context(nc.semaphore("ssem"))
        dsemb = [
            stack.enter_context(nc.semaphore(f"dsem{b}")) for b in range(NBUF_IN)
        ]
        osemb = [
            stack.enter_context(nc.semaphore(f"osem{b}")) for b in range(NBUF_OB)
        ]
        mv_sb = stack.enter_context(nc.sbuf_tensor([KDIM, NBUF_IN, CHUNK, 64], f16))
        kb_sb = stack.enter_context(nc.sbuf_tensor([KDIM, NBUF_IN, CHUNK, MDIM], f16))
        ob_sb = stack.enter_context(nc.sbuf_tensor([MDIM, NBUF_OB, CHUNK, 64], f16))
        warm_sb = stack.enter_context(nc.sbuf_tensor([MDIM, 8], f16))
        ps = stack.enter_context(nc.psum_tensor([MDIM, 2, CHUNK, 64], f32))

        @block.sync
        def _(sync):
            for c in range(NCHUNK):
                if c >= NBUF_IN:
                    # input slot c%NBUF_IN reused -> PE must be done with it
                    sync.wait_ge(tsem, c - NBUF_IN + 1)
                b = c % NBUF_IN
                sync.dma_start(out=mv_sb[:, b], in_=mv_ext[:, c]).then_inc(dsemb[b], 16)
                sync.dma_start(out=kb_sb[:, b], in_=kb_ext[:, c]).then_inc(dsemb[b], 16)
                if c >= 2:
                    oc = c - 2
                    sync.wait_ge(vsem, oc + 1)
                    sync.wait_ge(ssem, oc + 1)
                    sync.dma_start(
                        out=o_ext[:, oc], in_=ob_sb[:, oc % NBUF_OB]
                    ).then_inc(osemb[oc % NBUF_OB], 16)
            for oc in (NCHUNK - 2, NCHUNK - 1):
                sync.wait_ge(vsem, oc + 1)
                sync.wait_ge(ssem, oc + 1)
                sync.dma_start(
                    out=o_ext[:, oc], in_=ob_sb[:, oc % NBUF_OB]
                ).then_inc(osemb[oc % NBUF_OB], 16)

        @block.tensor
        def _(t):
            for c in range(NCHUNK):
                t.wait_ge(dsemb[c % NBUF_IN], 32 * (c // NBUF_IN + 1))
                if c >= 2:
                    # PSUM buffer c%2 reused -> drains of chunk c-2 done
                    t.wait_ge(vsem, c - 1)
                    t.wait_ge(ssem, c - 1)
                b = c % NBUF_IN
                pb = c % 2
                last = None
                for i in range(CHUNK):
                    last = t.matmul(
                        ps[:, pb, i], kb_sb[:, b, i], mv_sb[:, b, i],
                        start=True, stop=True,
                    )
                last.then_inc(tsem, 1)

        @block.vector
        def _(v):
            for c in range(NCHUNK):
                v.wait_ge(tsem, c + 1)
                if c >= NBUF_OB:
                    # output slot c%NBUF_OB reused -> its last flush done
                    v.wait_ge(osemb[c % NBUF_OB], 16 * (c // NBUF_OB))
                v.tensor_scalar(
                    out=ob_sb[:, c % NBUF_OB, 0 : CHUNK // 2],
                    in0=ps[:, c % 2, 0 : CHUNK // 2],
                    scalar1=1.0, scalar2=None, op0=mybir.AluOpType.mult,
                ).then_inc(vsem, 1)

        @block.scalar
        def _(s):
            # warm the activation table before the pipeline starts
            s.activation(
                out=warm_sb[:], in_=warm_sb[:],
                func=mybir.ActivationFunctionType.Copy,
            )
            for c in range(NCHUNK):
                s.wait_ge(tsem, c + 1)
                if c >= NBUF_OB:
                    s.wait_ge(osemb[c % NBUF_OB], 16 * (c // NBUF_OB))
                s.activation(
                    out=ob_sb[:, c % NBUF_OB, CHUNK // 2 : CHUNK],
                    in_=ps[:, c % 2, CHUNK // 2 : CHUNK],
                    func=mybir.ActivationFunctionType.Copy,
                ).then_inc(ssem, 1)

    return nc


_NC = None
_HOOK_DONE = False
_IDX = None


def _install_ntff_hook():
    # bass_utils' trace path fetches the NTFF profile hook via
    # antenv.axon_hooks, which this image lacks. Install a shim and
    # register the ctypes-based hook (mirrors trn_boot.boot()).
    global _HOOK_DONE
    if _HOOK_DONE:
        return
    _HOOK_DONE = True
    try:
        import antenv.axon_hooks  # noqa: F401

        return
    except ImportError:
        pass
    try:
        import contextlib
        import ctypes
        import types

        import antenv

        mod = types.ModuleType("antenv.axon_hooks")
        holder = {"hook": None}
        mod.set_axon_ntff_profile_hook = lambda h: holder.__setitem__("hook", h)
        mod.get_axon_ntff_profile_hook = lambda: holder["hook"]
        sys.modules["antenv.axon_hooks"] = mod
        antenv.axon_hooks = mod

        lib = ctypes.CDLL("/opt/axon/libaxon_pjrt.so")
        if not hasattr(lib, "axon_start_nrt_profile"):
            return
        lib.axon_start_nrt_profile.argtypes = [
            ctypes.POINTER(ctypes.c_int64),
            ctypes.c_size_t,
        ]
        lib.axon_start_nrt_profile.restype = ctypes.c_int64
        lib.axon_stop_nrt_profile.argtypes = [ctypes.c_char_p]
        lib.axon_stop_nrt_profile.restype = ctypes.c_int64

        @contextlib.contextmanager
        def _hook(output_dir, device_ids):
            import jax

            jax.devices()
            if device_ids:
                ids = (ctypes.c_int64 * len(device_ids))(*device_ids)
                rc = lib.axon_start_nrt_profile(ids, len(device_ids))
            else:
                rc = lib.axon_start_nrt_profile(None, 0)
            if rc != 0:
                raise RuntimeError(f"axon_start_nrt_profile rc={rc}")
            try:
                yield
            finally:
                n = lib.axon_stop_nrt_profile(str(output_dir).encode())
                if n < 0:
                    raise RuntimeError(f"axon_stop_nrt_profile rc={n}")

        mod.set_axon_ntff_profile_hook(_hook)

        from concourse import bass_utils as _bu

        _bu.upload_artifacts = lambda tmpdir: "local://" + str(tmpdir)
    except Exception:
        pass


def _patch_index():
    # mv gather index [KDIM, GROUPS]: row (i,t) of group g reads padded-FT
    # linear row (h + t%3)*130 + (w + t//3) for pixel px = g*14+i.
    global _IDX
    if _IDX is not None:
        return _IDX
    px = np.arange(GROUPS * GPC)
    px = np.minimum(px, 16383)
    h, w = px // 128, px % 128
    t = np.arange(TAPS)
    dy, dx = t % 3, t // 3
    # [GROUPS*GPC, TAPS]
    lin = (h[:, None] + dy[None, :]) * 130 + (w[:, None] + dx[None, :])
    idx = lin.reshape(GROUPS, GPC, TAPS).transpose(1, 2, 0).reshape(KDIM, GROUPS)
    _IDX = np.ascontiguousarray(idx)
    return _IDX


def _prep_core_inputs(feat_n, kern_n):
    # feat_n (64,128,128) f32, kern_n (36,128,128) f32
    idx = _patch_index()
    ftp = np.zeros((130, 130, 64), np.float16)
    ftp[1:129, 1:129] = feat_n.transpose(1, 2, 0)
    mv = ftp.reshape(16900, 64)[idx]  # (KDIM, GROUPS, 64)

    kres = kern_n.reshape(TAPS, 4, 16384).astype(np.float16)
    kpad = np.zeros((TAPS, 4, GROUPS * GPC), np.float16)
    kpad[:, :, :16384] = kres
    # kv[i, t, g, s]
    kv = kpad.reshape(TAPS, 4, GROUPS, GPC).transpose(3, 0, 2, 1)
    kb5 = np.zeros((GPC, TAPS, GROUPS, GPC, 4), np.float16)
    ii = np.arange(GPC)
    kb5[ii, :, :, ii, :] = kv
    kb = kb5.reshape(KDIM, GROUPS, MDIM)
    return {
        "mv": np.ascontiguousarray(mv).reshape(KDIM, NCHUNK, CHUNK, 64),
        "kb": np.ascontiguousarray(kb).reshape(KDIM, NCHUNK, CHUNK, MDIM),
    }


def _assemble_output(raw_o):
    # raw_o (MDIM, NCHUNK, CHUNK, 64) f16 -> (64, 256, 256) f32
    o = raw_o.astype(np.float32).reshape(GPC, 4, GROUPS, 64)
    o = o.transpose(3, 2, 0, 1).reshape(64, GROUPS * GPC, 4)[:, :16384]
    o = o.reshape(64, 128, 128, 2, 2).transpose(0, 1, 3, 2, 4)
    return o.reshape(64, 256, 256)


def kernel(feature: np.ndarray, kernel: np.ndarray) -> np.ndarray:
    global _NC, LAST_EXEC_TIME_NS
    if _NC is None:
        _NC = _build_program()
    feature = np.asarray(feature, dtype=np.float32)
    kernel = np.asarray(kernel, dtype=np.float32)
    in_maps = [_prep_core_inputs(feature[n], kernel[n]) for n in range(N)]
    trace = os.environ.get("PIXELCONV_TRACE", "") not in ("", "0")
    if trace:
        _install_ntff_hook()
    res = run_bass_kernel_spmd(
        _NC, in_maps, core_ids=list(range(N)), trace=trace
    )
    LAST_EXEC_TIME_NS = getattr(res, "exec_time_ns", None)
    out = np.stack([_assemble_output(res.results[n]["o"]) for n in range(N)])
    return out.astype(np.float32)


# revision 7
# speedup vs baseline: 2.4654x; 1.2973x over previous
import os
import sys

sys.path.insert(0, "/opt/trn_rl_repo")

import numpy as np
from concourse import bass, mybir
from concourse.bass_utils import run_bass_kernel_spmd

# nn_PixelConv: feature (8,64,128,128) f32, kernel (8,36,128,128) f32
# -> out (8,64,256,256) f32.  out[n,c,2h+r,2w+q] =
#   sum_{dx,dy in 0..2} F[n,c,h+dy-1,w+dx-1] * K[n,(dx*3+dy)*4+r*2+q,h,w]
# Sharding: pure data-parallel, batch n -> core n.
#
# PE formulation: per pixel px, out[c, s] = sum_t patch[px, t, c] * K[px, t, s]
# (t = dx*3+dy, 9 taps; s = r*2+q, 4 subpixels).  Batch 14 pixels per matmul
# as a block-diagonal stationary:
#   lhsT[K=126=(i,t), M=56=(i,s)] = K-vals on the 14 diagonal 9x4 blocks
#   rhs [K=126,        N=64=c   ] = im2col patches (host-gathered)
#   out [M=56,         N=64    ]  = PSUM fp32, drained to f16 by DVE+Act.
#
# The stationary is 93% structural zeros, so only the 4 K-values per
# (pixel, tap) row are DMA'd (packed with the patches in one stream);
# the DVE expands them on device into the block-diagonal via one
# tensor_tensor against a constant 0/1 mask (Kc broadcast over the 14
# column blocks, mask kills the 13 wrong ones).
#
# DMA completion sems are per buffer slot: a single counting sem is
# ambiguous when several transfers are in flight (each incs 16 via
# independent per-engine sub-increments, so a threshold can be reached
# with an older transfer still incomplete).  Slot sems only ever carry
# increments from rounds <= the awaited one (issue order is gated on
# consumer progress), so their thresholds are exact.

N = 8
GPC = 14           # pixels per matmul group
TAPS = 9
KDIM = GPC * TAPS  # 126
MDIM = GPC * 4     # 56
GROUPS = 1216      # 38 * 32; 1216*14 = 17024 >= 16384
CHUNK = 32         # groups per PSUM chunk
NCHUNK = GROUPS // CHUNK  # 38 (even: DMA in chunk pairs)
NPAIR = NCHUNK // 2
DVE_G = 12         # groups per chunk drained by DVE (Act takes the rest)

LAST_EXEC_TIME_NS = None

f16 = mybir.dt.float16
f32 = mybir.dt.float32


def _build_program():
    nc = bass.Bass()
    # packed per-group input: 64 patch cols (c) + 4 K-values
    mk_ext = nc.dram_tensor("mk", [KDIM, NCHUNK, CHUNK, 68], f16, kind="ExternalInput")
    mask_ext = nc.dram_tensor("mask", [KDIM, GPC, 4], f16, kind="ExternalInput")
    o_ext = nc.dram_tensor("o", [MDIM, NCHUNK, CHUNK, 64], f16, kind="ExternalOutput")

    import contextlib

    with contextlib.ExitStack() as stack:
        block = stack.enter_context(nc.Block())
        tsem = stack.enter_context(nc.semaphore("tsem"))
        vsem = stack.enter_context(nc.semaphore("vsem"))
        ssem = stack.enter_context(nc.semaphore("ssem"))
        bsem = stack.enter_context(nc.semaphore("bsem"))
        msem = stack.enter_context(nc.semaphore("msem"))
        dsemb = [stack.enter_context(nc.semaphore(f"dsem{b}")) for b in range(2)]
        osemb = [stack.enter_context(nc.semaphore(f"osem{b}")) for b in range(2)]
        # 4 chunk slots = 2 pair slots
        mk_sb = stack.enter_context(nc.sbuf_tensor([KDIM, 4, CHUNK, 68], f16))
        kb_sb = stack.enter_context(nc.sbuf_tensor([KDIM, 2, CHUNK, GPC, 4], f16))
        ob_sb = stack.enter_context(nc.sbuf_tensor([MDIM, 4, CHUNK, 64], f16))
        mask_sb = stack.enter_context(nc.sbuf_tensor([KDIM, GPC, 4], f16))
        warm_sb = stack.enter_context(nc.sbuf_tensor([MDIM, 8], f16))
        ps = stack.enter_context(nc.psum_tensor([MDIM, 2, CHUNK, 64], f32))

        @block.sync
        def _(sync):
            sync.dma_start(out=mask_sb[:], in_=mask_ext[:]).then_inc(msem, 16)
            for p in range(NPAIR):
                c = 2 * p
                if p >= 2:
                    # chunk slots 2(p%2), +1 reused by chunks c-4, c-3
                    sync.wait_ge(tsem, c - 2)
                sync.dma_start(
                    out=mk_sb[:, 2 * (p % 2) : 2 * (p % 2) + 2],
                    in_=mk_ext[:, c : c + 2],
                ).then_inc(dsemb[p % 2], 16)
                if p >= 2:
                    oc = c - 4  # flush output pair (oc, oc+1)
                    sync.wait_ge(vsem, oc + 2)
                    sync.wait_ge(ssem, oc + 2)
                    sync.dma_start(
                        out=o_ext[:, oc : oc + 2],
                        in_=ob_sb[:, 2 * ((p - 2) % 2) : 2 * ((p - 2) % 2) + 2],
                    ).then_inc(osemb[(p - 2) % 2], 16)
            for p in (NPAIR - 2, NPAIR - 1):
                oc = 2 * p
                sync.wait_ge(vsem, oc + 2)
                sync.wait_ge(ssem, oc + 2)
                sync.dma_start(
                    out=o_ext[:, oc : oc + 2],
                    in_=ob_sb[:, 2 * (p % 2) : 2 * (p % 2) + 2],
                ).then_inc(osemb[p % 2], 16)

        @block.vector
        def _(v):
            # interleave block-diagonal builds (for PE) with PSUM drains
            v.wait_ge(msem, 16)
            for c in range(NCHUNK + 1):
                if c < NCHUNK:
                    # build chunk c: kb[p, g, j, s] = Kc[p, g, s] * mask[p, j, s]
                    v.wait_ge(dsemb[(c // 2) % 2], 16 * (c // 4 + 1))
                    if c >= 2:
                        v.wait_ge(tsem, c - 1)  # kb slot c%2 free
                    in1 = (
                        mk_sb[:, c % 4, :, 64:68]
                        .unsqueeze(2)
                        .broadcast_to([KDIM, CHUNK, GPC, 4])
                    )
                    in0 = (
                        mask_sb[:]
                        .unsqueeze(1)
                        .broadcast_to([KDIM, CHUNK, GPC, 4])
                    )
                    v.tensor_tensor(
                        out=kb_sb[:, c % 2], in0=in0, in1=in1,
                        op=mybir.AluOpType.mult,
                    ).then_inc(bsem, 1)
                if c >= 1:
                    dc = c - 1
                    v.wait_ge(tsem, dc + 1)
                    if dc >= 4:
                        v.wait_ge(osemb[(dc % 4) // 2], 16 * (dc // 4))
                    v.tensor_scalar(
                        out=ob_sb[:, dc % 4, 0:DVE_G],
                        in0=ps[:, dc % 2, 0:DVE_G],
                        scalar1=1.0, scalar2=None, op0=mybir.AluOpType.mult,
                    ).then_inc(vsem, 1)

        @block.tensor
        def _(t):
            for c in range(NCHUNK):
                t.wait_ge(bsem, c + 1)
                t.wait_ge(dsemb[(c // 2) % 2], 16 * (c // 4 + 1))
                if c >= 2:
                    # PSUM buffer c%2 reused -> drains of chunk c-2 done
                    t.wait_ge(vsem, c - 1)
                    t.wait_ge(ssem, c - 1)
                last = None
                for i in range(CHUNK):
                    last = t.matmul(
                        ps[:, c % 2, i], kb_sb[:, c % 2, i],
                        mk_sb[:, c % 4, i, 0:64],
                        start=True, stop=True,
                    )
                last.then_inc(tsem, 1)

        @block.scalar
        def _(s):
            # warm the activation table before the pipeline starts
            s.activation(
                out=warm_sb[:], in_=warm_sb[:],
                func=mybir.ActivationFunctionType.Copy,
            )
            for c in range(NCHUNK):
                s.wait_ge(tsem, c + 1)
                if c >= 4:
                    s.wait_ge(osemb[(c % 4) // 2], 16 * (c // 4))
                s.activation(
                    out=ob_sb[:, c % 4, DVE_G:CHUNK],
                    in_=ps[:, c % 2, DVE_G:CHUNK],
                    func=mybir.ActivationFunctionType.Copy,
                ).then_inc(ssem, 1)

    return nc


_NC = None
_HOOK_DONE = False
_IDX = None


def _install_ntff_hook():
    # bass_utils' trace path fetches the NTFF profile hook via
    # antenv.axon_hooks, which this image lacks. Install a shim and
    # register the ctypes-based hook (mirrors trn_boot.boot()).
    global _HOOK_DONE
    if _HOOK_DONE:
        return
    _HOOK_DONE = True
    try:
        import antenv.axon_hooks  # noqa: F401

        return
    except ImportError:
        pass
    try:
        import contextlib
        import ctypes
        import types

        import antenv

        mod = types.ModuleType("antenv.axon_hooks")
        holder = {"hook": None}
        mod.set_axon_ntff_profile_hook = lambda h: holder.__setitem__("hook", h)
        mod.get_axon_ntff_profile_hook = lambda: holder["hook"]
        sys.modules["antenv.axon_hooks"] = mod
        antenv.axon_hooks = mod

        lib = ctypes.CDLL("/opt/axon/libaxon_pjrt.so")
        if not hasattr(lib, "axon_start_nrt_profile"):
            return
        lib.axon_start_nrt_profile.argtypes = [
            ctypes.POINTER(ctypes.c_int64),
            ctypes.c_size_t,
        ]
        lib.axon_start_nrt_profile.restype = ctypes.c_int64
        lib.axon_stop_nrt_profile.argtypes = [ctypes.c_char_p]
        lib.axon_stop_nrt_profile.restype = ctypes.c_int64

        @contextlib.contextmanager
        def _hook(output_dir, device_ids):
            import jax

            jax.devices()
            if device_ids:
                ids = (ctypes.c_int64 * len(device_ids))(*device_ids)
                rc = lib.axon_start_nrt_profile(ids, len(device_ids))
            else:
                rc = lib.axon_start_nrt_profile(None, 0)
            if rc != 0:
                raise RuntimeError(f"axon_start_nrt_profile rc={rc}")
            try:
                yield
            finally:
                n = lib.axon_stop_nrt_profile(str(output_dir).encode())
                if n < 0:
                    raise RuntimeError(f"axon_stop_nrt_profile rc={n}")

        mod.set_axon_ntff_profile_hook(_hook)

        from concourse import bass_utils as _bu

        _bu.upload_artifacts = lambda tmpdir: "local://" + str(tmpdir)
    except Exception:
        pass


def _patch_index():
    # mv gather index [KDIM, GROUPS]: row (i,t) of group g reads padded-FT
    # linear row (h + t%3)*130 + (w + t//3) for pixel px = g*14+i.
    global _IDX
    if _IDX is not None:
        return _IDX
    px = np.arange(GROUPS * GPC)
    px = np.minimum(px, 16383)
    h, w = px // 128, px % 128
    t = np.arange(TAPS)
    dy, dx = t % 3, t // 3
    lin = (h[:, None] + dy[None, :]) * 130 + (w[:, None] + dx[None, :])
    idx = lin.reshape(GROUPS, GPC, TAPS).transpose(1, 2, 0).reshape(KDIM, GROUPS)
    _IDX = np.ascontiguousarray(idx)
    return _IDX


def _make_mask():
    m = np.zeros((GPC, TAPS, GPC, 4), np.float16)
    ii = np.arange(GPC)
    m[ii, :, ii, :] = 1.0
    return m.reshape(KDIM, GPC, 4)


def _prep_core_inputs(feat_n, kern_n):
    # feat_n (64,128,128) f32, kern_n (36,128,128) f32
    idx = _patch_index()
    ftp = np.zeros((130, 130, 64), np.float16)
    ftp[1:129, 1:129] = feat_n.transpose(1, 2, 0)
    mk = np.empty((KDIM, GROUPS, 68), np.float16)
    mk[:, :, 0:64] = ftp.reshape(16900, 64)[idx]

    kres = kern_n.reshape(TAPS, 4, 16384).astype(np.float16)
    kpad = np.zeros((TAPS, 4, GROUPS * GPC), np.float16)
    kpad[:, :, :16384] = kres
    # Kc[(i,t), g, s]
    mk[:, :, 64:68] = (
        kpad.reshape(TAPS, 4, GROUPS, GPC)
        .transpose(3, 0, 2, 1)
        .reshape(KDIM, GROUPS, 4)
    )
    return {
        "mk": np.ascontiguousarray(mk).reshape(KDIM, NCHUNK, CHUNK, 68),
        "mask": _make_mask(),
    }


def _assemble_output(raw_o):
    # raw_o (MDIM, NCHUNK, CHUNK, 64) f16 -> (64, 256, 256) f32
    o = raw_o.astype(np.float32).reshape(GPC, 4, GROUPS, 64)
    o = o.transpose(3, 2, 0, 1).reshape(64, GROUPS * GPC, 4)[:, :16384]
    o = o.reshape(64, 128, 128, 2, 2).transpose(0, 1, 3, 2, 4)
    return o.reshape(64, 256, 256)


def kernel(feature: np.ndarray, kernel: np.ndarray) -> np.ndarray:
    global _NC, LAST_EXEC_TIME_NS
    if _NC is None:
        _NC = _build_program()
    feature = np.asarray(feature, dtype=np.float32)
    kernel = np.asarray(kernel, dtype=np.float32)
    in_maps = [_prep_core_inputs(feature[n], kernel[n]) for n in range(N)]
    trace = os.environ.get("PIXELCONV_TRACE", "") not in ("", "0")
    if trace:
        _install_ntff_hook()
    res = run_bass_kernel_spmd(
        _NC, in_maps, core_ids=list(range(N)), trace=trace
    )
    LAST_EXEC_TIME_NS = getattr(res, "exec_time_ns", None)
    out = np.stack([_assemble_output(res.results[n]["o"]) for n in range(N)])
    return out.astype(np.float32)
